# revision 1
# baseline (speedup 1.0000x reference)
"""v2 Trainium2 Bass kernel (see kernel.py docstring for the overall scheme).

v2 changes vs v1:
 - Neuron-stage scores are built without any transpose/row-flatten: a
   broadcast stationary (every column = u') turns the moving W_q^T slices
   into Q-broadcast matrices directly in PSUM; q-bias is accumulated with a
   K=1 all-ones-row matmul from a host-supplied bias row; the k_j * rs scale
   is applied by per-partition tensor_scalar from the k columns; one big exp
   finishes E. This removes the 9.5us/batch SBUF->SBUF flatten and the
   per-neuron outer-product weight loads.
 - Bulk weight DMA rides the ACT HWDGE ring (nc.scalar) so the
   latency-critical chain DMAs (gather in/out) never queue behind 3MB
   prefetches on the SP ring.
 - Newton rsqrt: 2 iterations (seed rel err 3.4% -> ~4e-6 after 2).
"""
import sys
import numpy as np

sys.path.insert(0, "/opt/trn_rl_repo")

I, L, T, S = 128, 8, 128, 128
N_CORES = 8
TL = T // N_CORES
EPS = 1e-5
RS = float(1.0 / np.sqrt(np.float32(S)))
GC = 0.7978845608028654
GA = 0.044715
MAGIC = 0x5F3759DF

_cached = None


def _build():
    from concourse import bacc, tile, mybir

    fp32 = mybir.dt.float32
    int32 = mybir.dt.int32
    Exp = mybir.ActivationFunctionType.Exp
    Tanh = mybir.ActivationFunctionType.Tanh
    mul_op = mybir.AluOpType.mult
    add_op = mybir.AluOpType.add
    sub_op = mybir.AluOpType.subtract
    shr_op = mybir.AluOpType.arith_shift_right
    bypass = mybir.AluOpType.bypass

    nc = bacc.Bacc("TRN2", target_bir_lowering=False, debug=False,
                   enable_asserts=True, num_devices=N_CORES)

    tqkv_d = nc.dram_tensor("tqkv", [L, S, TL * 3 * S], fp32, kind="ExternalInput").ap()
    small_d = nc.dram_tensor("small", [L, S, 72], fp32, kind="ExternalInput").ap()
    # small cols: 0:32 kvbias | 32:48 wmt | 48:64 mt | 64:67 topo_c
    #             | 67:70 topo_bp | 70 gamma | 71 beta
    topo_wt_d = nc.dram_tensor("topo_wt", [L, S, 3 * S], fp32, kind="ExternalInput").ap()
    bqr_d = nc.dram_tensor("bqr", [L, TL * S], fp32, kind="ExternalInput").ap()
    pre_d = nc.dram_tensor("pre", [S, 18], fp32, kind="ExternalInput").ap()
    wbc_d = nc.dram_tensor("wbc", [TL, L], fp32, kind="ExternalInput").ap()
    ident_d = nc.dram_tensor("ident", [S, S], fp32, kind="ExternalInput").ap()
    magic_d = nc.dram_tensor("magic", [1, 2], int32, kind="ExternalInput").ap()
    out_d = nc.dram_tensor("out", [TL, 1], fp32, kind="ExternalOutput").ap()

    with tile.TileContext(nc) as tc:
        with tc.tile_pool(name="wpool", bufs=3) as wpool, \
             tc.tile_pool(name="spool", bufs=3) as spool, \
             tc.tile_pool(name="fixed", bufs=1) as fixed, \
             tc.tile_pool(name="work", bufs=1) as work, \
             tc.tile_pool(name="ps_big", bufs=1, space="PSUM") as ps_big, \
             tc.tile_pool(name="ps_kv", bufs=1, space="PSUM") as ps_kv, \
             tc.tile_pool(name="ps_sm", bufs=1, space="PSUM") as ps_sm:

            ident = fixed.tile([S, S], fp32)
            nc.scalar.dma_start(ident[:], ident_d)
            pre = fixed.tile([S, 18], fp32)
            nc.scalar.dma_start(pre[:], pre_d)
            wbc = fixed.tile([TL, L], fp32)
            nc.scalar.dma_start(wbc[:], wbc_d)
            magic = fixed.tile([1, 2], int32)
            nc.scalar.dma_start(magic[:], magic_d)
            ones_col = fixed.tile([S, 1], fp32)
            nc.vector.memset(ones_col[:], 1.0)
            ones_row = fixed.tile([1, S], fp32)
            nc.vector.memset(ones_row[:], 1.0)
            ones_mat = fixed.tile([S, S], fp32)
            nc.vector.memset(ones_mat[:], 1.0)

            v_col = work.tile([S, 1], fp32)
            u_col = work.tile([S, 1], fp32)
            up_col = work.tile([S, 1], fp32)
            sc = work.tile([1, 8], fp32)
            sci = sc[:].bitcast(int32)
            yA = work.tile([1, 1], fp32)
            yB = work.tile([1, 1], fp32)
            yAi = yA[:].bitcast(int32)
            yBi = yB[:].bitcast(int32)
            bc_sb = work.tile([S, 3], fp32)
            qkvt_c = work.tile([S, 3], fp32)
            qrow_t = work.tile([1, S], fp32)
            krow_t = work.tile([1, S], fp32)
            Et_sb = work.tile([S, S], fp32)
            pvr_t = work.tile([S, 2], fp32)
            nc.vector.memset(pvr_t[:], 1.0)
            Ub = work.tile([S, S], fp32)
            kvn = work.tile([S, 2 * TL], fp32)
            krs = work.tile([S, TL], fp32)
            sc_sb = work.tile([S, TL * S], fp32)
            E_sb = work.tile([S, TL * S], fp32)
            pvr_n = work.tile([S, 2 * TL], fp32)
            rden = work.tile([S, TL], fp32)
            zp = work.tile([S, TL], fp32)
            afr = work.tile([S, 2 * TL], fp32)
            aff_c = work.tile([TL, 1], fp32)

            scoresT = ps_big.tile([S, TL * S], fp32)      # 4 banks (Q bcast)
            kv_ps = ps_kv.tile([S, 2 * TL], fp32)         # 1 bank
            smps = ps_sm.tile([S, 512], fp32)             # 1 bank scratch
            A_ps = smps[:, 0:3]
            bc_ps = smps[:, 4:7]
            pvt_ps = smps[:, 8:10]
            sv_ps = smps[0:1, 10:11]
            svv_ps = smps[0:1, 11:12]
            afz_ps = smps[0:TL, 13:14]
            afu_ps = smps[0:TL, 14:15]
            pvn_ps = smps[:, 16:48]
            trq_ps = smps[0:1, 128:256]
            trk_ps = smps[0:1, 256:384]

            ag_in = []
            ag_out = []
            for b in range(L - 1):
                ag_in.append(tc.tile([TL, 1], fp32, space="DRAM",
                                     name=f"agin{b}")[0])
                ag_out.append(tc.tile([S, 1], fp32, space="DRAM",
                                      addr_space="Shared", name=f"agout{b}")[0])

            def ts(out, in0, s1, op0, s2=None, op1=None):
                if s2 is None:
                    nc.vector.tensor_scalar(out, in0, s1, None, op0)
                else:
                    nc.vector.tensor_scalar(out, in0, s1, s2, op0, op1)

            for b in range(L):
                tqkv = wpool.tile([S, TL * 3 * S], fp32, tag="tqkv")
                nc.scalar.dma_start(tqkv[:], tqkv_d[b])
                topo_wt = spool.tile([S, 3 * S], fp32, tag="topo_wt")
                nc.scalar.dma_start(topo_wt[:], topo_wt_d[b])
                small = spool.tile([S, 72], fp32, tag="small")
                nc.scalar.dma_start(small[:], small_d[b])
                bqr = spool.tile([1, TL * S], fp32, tag="bqr")
                nc.scalar.dma_start(bqr[:], bqr_d[b])
                kvbias = small[:, 0:32]
                wmt = small[:, 32:48]
                mt = small[:, 48:64]
                topo_c = small[:, 64:67]
                topo_bp = small[:, 67:70]
                gam = small[:, 70:71]
                bet = small[:, 71:72]

                # ---- acquire v (and apply previous batch's adaptive gelu) ----
                if b == 0:
                    nc.vector.tensor_copy(v_col[:], pre[:, 0:1])
                else:
                    vin = work.tile([S, 1], fp32, tag="vin")
                    nc.sync.dma_start(vin[:], ag_out[b - 1][:])
                    g0 = pre[:, 1 + b:2 + b]
                    g1h = pre[:, 9 + b:10 + b]
                    xg = work.tile([S, 1], fp32, tag="xg")
                    t1 = work.tile([S, 1], fp32, tag="t1")
                    nc.vector.tensor_mul(xg[:], vin[:], g0)
                    nc.vector.tensor_mul(t1[:], xg[:], xg[:])
                    nc.vector.tensor_mul(t1[:], t1[:], xg[:])
                    nc.vector.scalar_tensor_tensor(t1[:], t1[:], GA, xg[:],
                                                   mul_op, add_op)
                    nc.scalar.activation(t1[:], t1[:], Tanh, scale=GC)
                    nc.vector.scalar_tensor_tensor(t1[:], t1[:], 1.0, xg[:],
                                                   add_op, mul_op)
                    nc.vector.tensor_mul(v_col[:], t1[:], g1h)

                # ---- stats + Newton rsqrt (2 iters) ----
                nc.tensor.matmul(sv_ps, ones_col[:], v_col[:], start=True, stop=True)
                nc.tensor.matmul(svv_ps, v_col[:], v_col[:], start=True, stop=True)
                ts(sc[:, 0:1], sv_ps, 1.0 / S, mul_op)
                ts(sc[:, 1:2], svv_ps, 1.0 / S, mul_op)
                # sc3 = mu^2 - msq = -var
                nc.vector.scalar_tensor_tensor(sc[:, 3:4], sc[:, 0:1], sc[:, 0:1],
                                               sc[:, 1:2], mul_op, sub_op)
                ts(sc[:, 4:5], sc[:, 3:4], -1.0, mul_op, EPS, add_op)     # vpe
                ts(sc[:, 5:6], sc[:, 3:4], -0.5, mul_op, 0.5 * EPS, add_op)  # vh
                ts(yBi, sci[:, 4:5], 1, shr_op)
                nc.vector.tensor_sub(yAi, magic[:, 0:1], yBi)
                for _ in range(2):
                    nc.vector.scalar_tensor_tensor(yB[:], yA[:], sc[:, 5:6],
                                                   yA[:], mul_op, mul_op)
                    ts(yB[:], yB[:], -1.0, mul_op, 1.5, add_op)
                    nc.vector.tensor_mul(yA[:], yA[:], yB[:])
                nc.vector.tensor_copy(sc[:, 6:7], yA[:])
                nc.vector.tensor_mul(sc[:, 7:8], yA[:], sc[:, 0:1])
                nc.tensor.matmul(bc_ps, ones_row[:], sc[:, 5:8], start=True, stop=True)
                nc.vector.tensor_copy(bc_sb[:], bc_ps)
                rstd_c = bc_sb[:, 1:2]
                murstd_c = bc_sb[:, 2:3]

                # ---- u = rstd*gamma*(v-mu) + beta ----
                gv = work.tile([S, 1], fp32, tag="gv")
                gm = work.tile([S, 1], fp32, tag="gm")
                nc.vector.tensor_mul(gv[:], v_col[:], gam)
                ts(gm[:], gam, murstd_c, mul_op)
                nc.vector.scalar_tensor_tensor(u_col[:], gv[:], rstd_c, gm[:],
                                               mul_op, sub_op)
                nc.vector.tensor_add(u_col[:], u_col[:], bet)

                # ---- topo qkv on raw v (gamma folded into weights) ----
                for m in range(3):
                    nc.tensor.matmul(A_ps[:, m:m + 1], topo_wt[:, m * S:(m + 1) * S],
                                     v_col[:], start=True, stop=True)
                cm = work.tile([S, 3], fp32, tag="cm")
                ts(cm[:], topo_c, murstd_c, mul_op)
                nc.vector.scalar_tensor_tensor(qkvt_c[:], A_ps, rstd_c, cm[:],
                                               mul_op, sub_op)
                nc.vector.tensor_add(qkvt_c[:], qkvt_c[:], topo_bp)

                # ---- topo attention ----
                nc.tensor.transpose(trq_ps, qkvt_c[:, 0:1], ident[:])
                nc.tensor.transpose(trk_ps, qkvt_c[:, 1:2], ident[:])
                nc.vector.tensor_copy(qrow_t[:], trq_ps)
                nc.vector.tensor_copy(krow_t[:], trk_ps)
                nc.tensor.matmul(scoresT[:, 0:S], krow_t[:], qrow_t[:],
                                 start=True, stop=True)
                nc.scalar.activation(Et_sb[:], scoresT[:, 0:S], Exp, scale=RS)
                nc.vector.tensor_copy(pvr_t[:, 0:1], qkvt_c[:, 2:3])
                nc.tensor.matmul(pvt_ps, Et_sb[:], pvr_t[:], start=True, stop=True)
                rd1 = work.tile([S, 1], fp32, tag="rd1")
                nc.vector.reciprocal(rd1[:], pvt_ps[:, 1:2])
                nc.vector.tensor_mul(up_col[:], pvt_ps[:, 0:1], rd1[:])
                nc.vector.tensor_add(up_col[:], up_col[:], u_col[:])

                # ---- neuron k,v columns ----
                for tl in range(TL):
                    nc.tensor.matmul(kv_ps[:, 2 * tl:2 * tl + 1],
                                     tqkv[:, (3 * tl + 1) * S:(3 * tl + 2) * S],
                                     up_col[:], start=True, stop=True)
                    nc.tensor.matmul(kv_ps[:, 2 * tl + 1:2 * tl + 2],
                                     tqkv[:, (3 * tl + 2) * S:(3 * tl + 3) * S],
                                     up_col[:], start=True, stop=True)
                nc.vector.tensor_add(kvn[:], kv_ps[:], kvbias)
                k2 = kvn[:].rearrange("p (t k) -> p t k", k=2)
                ts(krs[:], k2[:, :, 0], RS, mul_op)
                p2 = pvr_n[:].rearrange("p (t k) -> p t k", k=2)
                nc.vector.tensor_mul(p2[:, :, 0], k2[:, :, 1], mt)
                nc.vector.tensor_copy(p2[:, :, 1], mt)

                # ---- Q broadcast (+ q bias) accumulated in PSUM ----
                nc.vector.tensor_scalar(Ub[:], ones_mat[:], up_col[:], None, mul_op)
                for bank in range(4):
                    for j in range(4):
                        tl = 4 * bank + j
                        nc.tensor.matmul(
                            scoresT[:, tl * S:(tl + 1) * S], Ub[:],
                            tqkv[:, 3 * tl * S:(3 * tl + 1) * S],
                            start=(j == 0), stop=False, skip_group_check=True)
                    nc.tensor.matmul(
                        scoresT[:, bank * 512:(bank + 1) * 512], ones_row[:],
                        bqr[:, bank * 512:(bank + 1) * 512],
                        start=False, stop=True, skip_group_check=True)

                # ---- scale by k*rs, exp (per bank, so PV overlaps), PV ----
                for bank in range(4):
                    for j in range(4):
                        tl = 4 * bank + j
                        ts(sc_sb[:, tl * S:(tl + 1) * S],
                           scoresT[:, tl * S:(tl + 1) * S], krs[:, tl:tl + 1],
                           mul_op)
                    nc.scalar.activation(E_sb[:, bank * 512:(bank + 1) * 512],
                                         sc_sb[:, bank * 512:(bank + 1) * 512],
                                         Exp)
                    for j in range(4):
                        tl = 4 * bank + j
                        nc.tensor.matmul(pvn_ps[:, 2 * tl:2 * tl + 2],
                                         E_sb[:, tl * S:(tl + 1) * S],
                                         pvr_n[:, 2 * tl:2 * tl + 2],
                                         start=True, stop=True)
                pv2 = pvn_ps.rearrange("p (t k) -> p t k", k=2)
                nc.vector.reciprocal(rden[:], pv2[:, :, 1])
                nc.vector.tensor_mul(zp[:], pv2[:, :, 0], rden[:])

                # ---- aff = sum_i wmt*(zp + u') + wbias ----
                nc.vector.tensor_mul(afr[:, 0:TL], wmt, zp[:])
                ts(afr[:, TL:2 * TL], wmt, up_col[:], mul_op)
                nc.tensor.matmul(afz_ps, afr[:, 0:TL], ones_col[:],
                                 start=True, stop=True)
                nc.tensor.matmul(afu_ps, afr[:, TL:2 * TL], ones_col[:],
                                 start=True, stop=True)
                nc.vector.tensor_copy(aff_c[:], afz_ps)
                nc.vector.tensor_add(aff_c[:], aff_c[:], afu_ps)
                nc.vector.tensor_add(aff_c[:], aff_c[:], wbc[:, b:b + 1])

                if b < L - 1:
                    nc.sync.dma_start(ag_in[b][:], aff_c[:])
                    nc.gpsimd.collective_compute(
                        "AllGather", bypass,
                        replica_groups=[list(range(N_CORES))],
                        ins=[ag_in[b].opt()], outs=[ag_out[b].opt()],
                    )
                else:
                    nc.sync.dma_start(out_d, aff_c[:])

    nc.compile()
    return nc


def _host_prep(x, W, mask, attn_t, attn_n, norm_params, ada):
    f32 = np.float32
    x, W, mask, attn_t, attn_n, norm_params, ada = (
        np.ascontiguousarray(np.asarray(a, f32))
        for a in (x, W, mask, attn_t, attn_n, norm_params, ada))
    gamma = norm_params[:, 0, :]
    beta = norm_params[:, 1, :]

    topo_w = attn_t[:, :, :, :S]
    topo_b = attn_t[:, :, :, S]
    topo_wg = topo_w * gamma[:, None, None, :]
    topo_wt_flat = np.ascontiguousarray(
        topo_wg.transpose(0, 3, 1, 2)).reshape(L, S, 3 * S)
    topo_c = topo_wg.sum(axis=3)
    topo_bp = np.einsum('lmis,ls->lmi', topo_w, beta) + topo_b

    wmat = W[:, :, :S] * mask
    wbias = W[:, :, S]

    pre = np.zeros((S, 18), f32)
    pre[:, 0] = x
    pre[:, 2:10] = ada[:, :, 0].T
    pre[:, 10:18] = (0.5 * ada[:, :, 1]).astype(f32).T

    ident = np.eye(S, dtype=f32)
    magic = np.array([[MAGIC, 0]], np.int32)

    in_maps = []
    for c in range(N_CORES):
        sl = slice(c * TL, (c + 1) * TL)
        an = attn_n[:, sl]
        anw = an[:, :, :, :, :S]
        anb = an[:, :, :, :, S]                              # (L,TL,3,i)
        tqkv = np.ascontiguousarray(
            anw.transpose(0, 4, 1, 2, 3)).reshape(L, S, TL * 3 * S)
        small = np.zeros((L, S, 72), f32)
        kv = np.stack([anb[:, :, 1, :], anb[:, :, 2, :]], axis=2)  # (L,TL,2,i)
        small[:, :, 0:32] = kv.transpose(0, 3, 1, 2).reshape(L, S, 2 * TL)
        small[:, :, 32:48] = wmat[:, sl].transpose(0, 2, 1)
        small[:, :, 48:64] = mask[:, sl].transpose(0, 2, 1)
        small[:, :, 64:67] = topo_c.transpose(0, 2, 1)
        small[:, :, 67:70] = topo_bp.transpose(0, 2, 1)
        small[:, :, 70] = gamma
        small[:, :, 71] = beta
        bqr = np.ascontiguousarray(anb[:, :, 0, :].reshape(L, TL * S))
        wbc = np.ascontiguousarray(wbias[:, sl].T)
        in_maps.append(dict(tqkv=tqkv, small=small, topo_wt=topo_wt_flat,
                            bqr=bqr, pre=pre, wbc=wbc, ident=ident, magic=magic))
    return in_maps


def kernel(x, W, mask, attn_t, attn_n, attn_mask_n, norm_params, ada,
           span_ids, tb_ids):
    global _cached
    from concourse import bass_utils
    if _cached is None:
        _cached = _build()
    nc = _cached
    in_maps = _host_prep(x, W, mask, attn_t, attn_n, norm_params, ada)
    res = bass_utils.run_bass_kernel_spmd(nc, in_maps, core_ids=list(range(N_CORES)))
    out = np.concatenate([res.results[c]["out"].reshape(TL) for c in range(N_CORES)])
    return out.astype(np.float32)



# revision 3
# speedup vs baseline: 1.0108x; 1.0108x over previous
"""v3 Trainium2 Bass kernel.

Scheme (per topo batch b, SPMD over 8 cores, core c owns neurons
[c*16, (c+1)*16) of each batch):
  - v arrives as the DRAM AllGather of the previous batch's per-core
    affine outputs; gelu/ada applied on the gathered column.
  - topo norm stats via two 1-col matmuls + fast-inverse-sqrt Newton
    (2 iters) on [1,1] scalars; rstd/mu*rstd broadcast to columns with
    one ones-row matmul.
  - topo + neuron attention scores are built as PURE OUTER-PRODUCT
    matmuls: q and k*rs are computed as PSUM columns (stationary =
    bf16 weights, moving = 1-col activations, ~2ns each), bias-added,
    transposed once ([S,32] -> [32,S] via one bf16 ident matmul), and
    each neuron's score tile is stat=krs_row (x) moving=q_row
    (~107ns/neuron, bf16).  No broadcast-weight streams, no separate
    bias matmuls, no DVE scale pass: exp reads raw PSUM scores.
  - rs is folded into the K weights/biases host-side; gamma into the
    topo weights; v-bias*mask into a host column.
  - All PE operands bf16 (weights DMA'd in bf16: halves HBM traffic,
    4x fewer PE cycles per moving column).
  - exp per 512-col PSUM bank on ACT, E in bf16; PV = stat=E slice,
    moving = interleaved [v*m | m] bf16 pairs.
  - affine = ones-col matmul over wm*(zp+u') + bias row; result DMA'd
    to DRAM and AllGather'd (the 15us collective constant dominates
    each boundary; everything else is packed around it).
"""
import sys
import numpy as np
import ml_dtypes

sys.path.insert(0, "/opt/trn_rl_repo")

I, L, T, S = 128, 8, 128, 128
N_CORES = 8
TL = T // N_CORES
EPS = 1e-5
RS = float(1.0 / np.sqrt(np.float32(S)))
GC = 0.7978845608028654
GA = 0.044715
MAGIC = 0x5F3759DF

_cached = None

# small column layout
C_QB = 0          # 0:16   q bias cols
C_KB = 16         # 16:32  k bias cols (x rs)
C_MT = 32         # 32:48  mask cols
C_BVMT = 48       # 48:64  v-bias*mask cols
C_WMT = 64        # 64:80  (W*mask) cols
C_TC = 80         # 80:83  topo_c (k row x rs)
C_TBP = 83        # 83:86  topo_bp
C_GAM = 86
C_BET = 87
C_G0 = 88         # ada[b-1,:,0]
C_G1H = 89        # 0.5*ada[b-1,:,1]
NC_SM = 90


def _build():
    from concourse import bacc, tile, mybir

    fp32 = mybir.dt.float32
    bf16 = mybir.dt.bfloat16
    int32 = mybir.dt.int32
    Exp = mybir.ActivationFunctionType.Exp
    Tanh = mybir.ActivationFunctionType.Tanh
    mul_op = mybir.AluOpType.mult
    add_op = mybir.AluOpType.add
    sub_op = mybir.AluOpType.subtract
    shr_op = mybir.AluOpType.arith_shift_right
    bypass = mybir.AluOpType.bypass
    div_op = mybir.AluOpType.divide

    nc = bacc.Bacc("TRN2", target_bir_lowering=False, debug=False,
                   enable_asserts=True, num_devices=N_CORES)

    tqkv_d = nc.dram_tensor("tqkv", [L, S, TL * 3 * S], bf16,
                            kind="ExternalInput").ap()
    topo_wt_d = nc.dram_tensor("topo_wt", [L, S, 3 * S], bf16,
                               kind="ExternalInput").ap()
    small_d = nc.dram_tensor("small", [L, S, NC_SM], fp32,
                             kind="ExternalInput").ap()
    pre_d = nc.dram_tensor("pre", [S, 1], fp32, kind="ExternalInput").ap()
    wbc_d = nc.dram_tensor("wbc", [TL, L], fp32, kind="ExternalInput").ap()
    ident_d = nc.dram_tensor("ident", [S, S], bf16, kind="ExternalInput").ap()
    selb_d = nc.dram_tensor("selb", [TL, TL * S], bf16, kind="ExternalInput").ap()
    magic_d = nc.dram_tensor("magic", [1, 2], int32, kind="ExternalInput").ap()
    out_d = nc.dram_tensor("out", [TL, 1], fp32, kind="ExternalOutput").ap()

    with tile.TileContext(nc) as tc:
        with tc.tile_pool(name="wpool", bufs=3) as wpool, \
             tc.tile_pool(name="spool", bufs=3) as spool, \
             tc.tile_pool(name="fixed", bufs=1) as fixed, \
             tc.tile_pool(name="work", bufs=1) as work, \
             tc.tile_pool(name="ps_big", bufs=1, space="PSUM") as ps_big, \
             tc.tile_pool(name="ps_sc", bufs=2, space="PSUM") as ps_sc:

            identb = fixed.tile([S, S], bf16)
            nc.scalar.dma_start(identb[:], ident_d)
            selb = fixed.tile([TL, TL * S], bf16)
            nc.scalar.dma_start(selb[:], selb_d)
            pre = fixed.tile([S, 1], fp32)
            nc.scalar.dma_start(pre[:], pre_d)
            wbc = fixed.tile([TL, L], fp32)
            nc.scalar.dma_start(wbc[:], wbc_d)
            magic = fixed.tile([1, 2], int32)
            nc.scalar.dma_start(magic[:], magic_d)

            ones_col = fixed.tile([S, 1], fp32)
            nc.vector.memset(ones_col[:], 1.0)
            ones_colb = fixed.tile([S, 1], bf16)
            nc.vector.memset(ones_colb[:], 1.0)
            ones_row = fixed.tile([1, S], fp32)
            nc.vector.memset(ones_row[:], 1.0)
            ones_rowb = fixed.tile([1, S], bf16)
            nc.vector.memset(ones_rowb[:], 1.0)
            ones_matb = fixed.tile([S, S], bf16)
            nc.vector.memset(ones_matb[:], 1.0)

            v_bf = work.tile([S, 1], bf16)
            u_col = work.tile([S, 1], fp32)
            sc = work.tile([1, 8], fp32)
            sci = sc[:].bitcast(int32)
            yA = work.tile([1, 1], fp32)
            yB = work.tile([1, 1], fp32)
            yAi = yA[:].bitcast(int32)
            yBi = yB[:].bitcast(int32)
            bc_sb = work.tile([S, 2], fp32)
            qkvt = work.tile([S, 3], fp32)
            qb_t = work.tile([S, 1], bf16)
            qrow_t = work.tile([1, S], bf16)
            Et_sb = work.tile([S, S], bf16)
            pvr_t = work.tile([S, 2], bf16)
            nc.vector.memset(pvr_t[:], 1.0)
            rd1 = work.tile([S, 1], fp32)
            up_f = work.tile([S, 1], fp32)
            up_bf = work.tile([S, 1], bf16)
            qkb = work.tile([S, TL], bf16)
            krs_f = work.tile([S, TL], fp32)
            qT = work.tile([TL, S], bf16)
            sc_sb = work.tile([S, TL * S], fp32)
            vm1 = work.tile([S, TL], fp32)
            pvr = work.tile([S, 2 * TL], bf16)
            pvr2 = pvr[:].rearrange("p (t k) -> p t k", k=2)
            E_sb = work.tile([S, TL * S], bf16)
            rden = work.tile([S, TL], fp32)
            zp = work.tile([S, TL], fp32)
            zpu = work.tile([S, TL], fp32)
            afr = work.tile([S, TL], fp32)
            aff_sb = work.tile([TL, 1], fp32)

            sbank = [ps_big.tile([S, 512], fp32, name=f"sbank{i}")
                     for i in range(4)]

            ag_in = []
            ag_out = []
            for b in range(L - 1):
                ag_in.append(tc.tile([TL, 1], fp32, space="DRAM",
                                     name=f"agin{b}")[0])
                ag_out.append(tc.tile([S, 1], fp32, space="DRAM",
                                      addr_space="Shared", name=f"agout{b}")[0])

            def ts(out, in0, s1, op0, s2=None, op1=None):
                if s2 is None:
                    nc.vector.tensor_scalar(out, in0, s1, None, op0)
                else:
                    nc.vector.tensor_scalar(out, in0, s1, s2, op0, op1)

            def load_weights(b, eng, split=False):
                tq = wpool.tile([S, TL * 3 * S], bf16, tag="tq")
                if split:
                    HC = TL * 3 * S // 2
                    nc.scalar.dma_start(tq[:, 0:HC], tqkv_d[b][:, 0:HC])
                    eng.dma_start(tq[:, HC:2 * HC], tqkv_d[b][:, HC:2 * HC])
                else:
                    eng.dma_start(tq[:], tqkv_d[b])
                tw = spool.tile([S, 3 * S], bf16, tag="tw")
                eng.dma_start(tw[:], topo_wt_d[b])
                sm = spool.tile([S, NC_SM], fp32, tag="sm")
                eng.dma_start(sm[:], small_d[b])
                return tq, tw, sm

            wtiles = load_weights(0, nc.gpsimd, split=True)
            for b in range(L):
                tq, tw, sm = wtiles

                scr = ps_sc.tile([S, 512], fp32, tag="scr")
                qkv_ps = scr[:, 0:48]
                pvn_ps = scr[:, 48:80]
                aff_ps = scr[0:TL, 80:81]
                sv_ps = scr[0:1, 96:97]
                svv_ps = scr[0:1, 97:98]
                bc_ps = scr[:, 100:102]
                A_ps = scr[:, 104:107]
                pvt_ps = scr[:, 110:112]
                tr_ps = scr[0:TL, 128:256]
                trt_ps = scr[0:1, 256:384]
                tsc_ps = scr[:, 384:512]

                gam = sm[:, C_GAM:C_GAM + 1]
                bet = sm[:, C_BET:C_BET + 1]

                # ---- acquire v (gelu of previous batch's outputs) ----
                if b == 0:
                    nc.vector.tensor_copy(v_bf[:], pre[:, 0:1])
                else:
                    vin = work.tile([S, 1], fp32, tag="vin")
                    nc.sync.dma_start(vin[:], ag_out[b - 1][:])
                    g0 = sm[:, C_G0:C_G0 + 1]
                    g1h = sm[:, C_G1H:C_G1H + 1]
                    xg = work.tile([S, 1], fp32, tag="xg")
                    t1 = work.tile([S, 1], fp32, tag="t1")
                    t2 = work.tile([S, 1], fp32, tag="t2")
                    wg = work.tile([S, 1], fp32, tag="wg")
                    ts(xg[:], vin[:], g0, mul_op)
                    nc.vector.tensor_mul(t1[:], xg[:], xg[:])
                    nc.vector.tensor_mul(t2[:], t1[:], xg[:])
                    nc.vector.scalar_tensor_tensor(t2[:], t2[:], GA, xg[:],
                                                   mul_op, add_op)
                    nc.scalar.activation(t2[:], t2[:], Tanh, scale=GC)
                    ts(wg[:], xg[:], g1h, mul_op)
                    nc.vector.scalar_tensor_tensor(v_bf[:], t2[:], 1.0, wg[:],
                                                   add_op, mul_op)

                # ---- stats + Newton rsqrt (PE + DVE) ----
                nc.tensor.matmul(sv_ps, ones_colb[:], v_bf[:],
                                 start=True, stop=True)
                nc.tensor.matmul(svv_ps, v_bf[:], v_bf[:],
                                 start=True, stop=True)
                ts(sc[:, 0:1], sv_ps, 1.0 / S, mul_op)
                ts(sc[:, 1:2], svv_ps, 1.0 / S, mul_op)
                nc.vector.scalar_tensor_tensor(sc[:, 3:4], sc[:, 0:1],
                                               sc[:, 0:1], sc[:, 1:2],
                                               mul_op, sub_op)
                ts(sc[:, 4:5], sc[:, 3:4], -1.0, mul_op, EPS, add_op)
                ts(sc[:, 5:6], sc[:, 3:4], -0.5, mul_op, 0.5 * EPS, add_op)
                ts(yBi, sci[:, 4:5], 1, shr_op)
                nc.vector.tensor_sub(yAi, magic[:, 0:1], yBi)
                for _ in range(2):
                    nc.vector.scalar_tensor_tensor(yB[:], yA[:], sc[:, 5:6],
                                                   yA[:], mul_op, mul_op)
                    ts(yB[:], yB[:], -1.0, mul_op, 1.5, add_op)
                    nc.vector.tensor_mul(yA[:], yA[:], yB[:])
                nc.vector.tensor_copy(sc[:, 6:7], yA[:])
                nc.vector.tensor_mul(sc[:, 7:8], yA[:], sc[:, 0:1])
                nc.tensor.matmul(bc_ps, ones_row[:], sc[:, 6:8],
                                 start=True, stop=True)
                rstd_c = bc_ps[:, 0:1]
                murstd_c = bc_ps[:, 1:2]

                # ---- u = rstd*gamma*(v-mu) + beta ----
                gv = work.tile([S, 1], fp32, tag="gv")
                gm2 = work.tile([S, 1], fp32, tag="gm2")
                ts(gv[:], v_bf[:], gam, mul_op)
                ts(gm2[:], gam, murstd_c, mul_op, bet, sub_op)
                nc.vector.scalar_tensor_tensor(u_col[:], gv[:], rstd_c,
                                               gm2[:], mul_op, sub_op)

                # ---- topo qkv (gamma, rs folded host-side) ----
                for m in range(3):
                    nc.tensor.matmul(A_ps[:, m:m + 1], tw[:, m * S:(m + 1) * S],
                                     v_bf[:], start=True, stop=True)
                cm2 = work.tile([S, 3], fp32, tag="cm2")
                nc.vector.scalar_tensor_tensor(cm2[:], sm[:, C_TC:C_TC + 3],
                                               murstd_c, sm[:, C_TBP:C_TBP + 3],
                                               mul_op, sub_op)
                nc.vector.scalar_tensor_tensor(qkvt[:, 1:3], A_ps[:, 1:3],
                                               rstd_c, cm2[:, 1:3],
                                               mul_op, sub_op)
                qc_t = work.tile([S, 1], fp32, tag="qc_t")
                nc.vector.scalar_tensor_tensor(qc_t[:], A_ps[:, 0:1], rstd_c,
                                               cm2[:, 0:1], mul_op, sub_op)

                # ---- topo attention: q bcast via ones-mat, krs exp scale ----
                Ub_t = work.tile([S, S], bf16, tag="Ub_t")
                nc.vector.tensor_scalar(Ub_t[:], ones_matb[:], qc_t[:],
                                        None, mul_op)
                nc.vector.tensor_copy(pvr_t[:, 0:1], qkvt[:, 2:3])
                nc.tensor.matmul(tsc_ps, Ub_t[:], identb[:],
                                 start=True, stop=True)
                nc.scalar.activation(Et_sb[:], tsc_ps, Exp,
                                     scale=qkvt[:, 1:2])
                nc.tensor.matmul(pvt_ps, Et_sb[:], pvr_t[:],
                                 start=True, stop=True)
                nc.vector.reciprocal(rd1[:], pvt_ps[:, 1:2])
                nc.vector.scalar_tensor_tensor(up_bf[:], pvt_ps[:, 0:1],
                                               rd1[:], u_col[:],
                                               mul_op, add_op)
                nc.vector.scalar_tensor_tensor(up_f[:], pvt_ps[:, 0:1],
                                               rd1[:], u_col[:],
                                               mul_op, add_op)

                # ---- neuron q,k,v columns ----
                for tl in range(TL):
                    nc.tensor.matmul(qkv_ps[:, tl:tl + 1],
                                     tq[:, 3 * tl * S:(3 * tl + 1) * S],
                                     up_bf[:], start=True, stop=True)
                for tl in range(TL):
                    nc.tensor.matmul(qkv_ps[:, 16 + tl:17 + tl],
                                     tq[:, (3 * tl + 1) * S:(3 * tl + 2) * S],
                                     up_bf[:], start=True, stop=True)
                for tl in range(TL):
                    nc.tensor.matmul(qkv_ps[:, 32 + tl:33 + tl],
                                     tq[:, (3 * tl + 2) * S:(3 * tl + 3) * S],
                                     up_bf[:], start=True, stop=True)
                nc.vector.tensor_add(qkb[:], qkv_ps[:, 0:TL],
                                     sm[:, C_QB:C_QB + TL])
                nc.tensor.matmul(tr_ps, qkb[:], identb[:],
                                 start=True, stop=True)
                nc.vector.tensor_copy(qT[:], tr_ps)
                nc.vector.tensor_add(krs_f[:], qkv_ps[:, TL:2 * TL],
                                     sm[:, C_KB:C_KB + TL])
                nc.vector.tensor_mul(vm1[:], qkv_ps[:, 32:48],
                                     sm[:, C_MT:C_MT + TL])
                nc.vector.tensor_add(pvr2[:, :, 0], vm1[:],
                                     sm[:, C_BVMT:C_BVMT + TL])
                nc.vector.tensor_copy(pvr2[:, :, 1], sm[:, C_MT:C_MT + TL])

                # ---- scores: selector q-bcast; krs folded into the exp's
                # per-partition scale; PV per tl ----
                for tl in range(TL):
                    nc.tensor.matmul(sbank[tl // 4][:, (tl % 4) * S:(tl % 4 + 1) * S],
                                     selb[:, tl * S:(tl + 1) * S],
                                     qT[:], start=True, stop=True)
                for tl in range(8):
                    nc.scalar.activation(E_sb[:, tl * S:(tl + 1) * S],
                                         sbank[tl // 4][:, (tl % 4) * S:(tl % 4 + 1) * S],
                                         Exp, scale=krs_f[:, tl:tl + 1])
                for tl in range(8, TL):
                    ts(sc_sb[:, tl * S:(tl + 1) * S],
                       sbank[tl // 4][:, (tl % 4) * S:(tl % 4 + 1) * S],
                       krs_f[:, tl:tl + 1], mul_op)
                for bank in range(2, 4):
                    nc.scalar.activation(E_sb[:, bank * 512:(bank + 1) * 512],
                                         sc_sb[:, bank * 512:(bank + 1) * 512],
                                         Exp)
                for tl in range(TL):
                    nc.tensor.matmul(pvn_ps[:, 2 * tl:2 * tl + 2],
                                     E_sb[:, tl * S:(tl + 1) * S],
                                     pvr[:, 2 * tl:2 * tl + 2],
                                     start=True, stop=True)

                # ---- affine + output ----
                pv2 = pvn_ps.rearrange("p (t k) -> p t k", k=2)
                nc.vector.reciprocal(rden[:], pv2[:, :, 1])
                nc.vector.tensor_mul(zp[:], pv2[:, :, 0], rden[:])
                ts(zpu[:], zp[:], up_f[:], add_op)
                nc.vector.tensor_mul(afr[:], zpu[:], sm[:, C_WMT:C_WMT + TL])
                nc.tensor.matmul(aff_ps, afr[:], ones_col[:],
                                 start=True, stop=True)
                nc.vector.tensor_add(aff_sb[:], aff_ps,
                                     wbc[:, b:b + 1])

                if b < L - 1:
                    nc.sync.dma_start(ag_in[b][:], aff_sb[:])
                    nc.gpsimd.collective_compute(
                        "AllGather", bypass,
                        replica_groups=[list(range(N_CORES))],
                        ins=[ag_in[b].opt()], outs=[ag_out[b].opt()],
                    )
                    wtiles = load_weights(b + 1, nc.gpsimd)
                else:
                    nc.sync.dma_start(out_d, aff_sb[:])

    nc.compile()
    return nc


def _host_prep(x, W, mask, attn_t, attn_n, norm_params, ada):
    f32 = np.float32
    bf = ml_dtypes.bfloat16
    x, W, mask, attn_t, attn_n, norm_params, ada = (
        np.ascontiguousarray(np.asarray(a, f32))
        for a in (x, W, mask, attn_t, attn_n, norm_params, ada))
    gamma = norm_params[:, 0, :]
    beta = norm_params[:, 1, :]

    rs_vec = np.array([1.0, RS, 1.0], f32)[None, :, None, None]  # scale k rows
    topo_w = attn_t[:, :, :, :S]
    topo_b = attn_t[:, :, :, S]
    topo_wg = topo_w * gamma[:, None, None, :] * rs_vec
    topo_wt_flat = np.ascontiguousarray(
        topo_wg.transpose(0, 3, 1, 2)).reshape(L, S, 3 * S).astype(bf)
    topo_c = topo_wg.sum(axis=3)                                   # (L,3,S)
    topo_bp = (np.einsum('lmis,ls->lmi', topo_w, beta) + topo_b) \
        * rs_vec[:, :, :, 0]

    wmat = W[:, :, :S] * mask
    wbias = W[:, :, S]

    ident = np.eye(S, dtype=bf)
    selb = np.zeros((TL, TL * S), dtype=bf)
    for tl in range(TL):
        selb[tl, tl * S:(tl + 1) * S] = 1.0
    magic = np.array([[MAGIC, 0]], np.int32)
    pre = np.ascontiguousarray(x.reshape(S, 1))

    in_maps = []
    for c in range(N_CORES):
        sl = slice(c * TL, (c + 1) * TL)
        an = attn_n[:, sl]
        anw = an[:, :, :, :, :S] * rs_vec[:, None, :, :, 0, None]  # (L,TL,3,j,i)
        anb = an[:, :, :, :, S] * rs_vec[:, None, :, :, 0]         # (L,TL,3,j)
        tqkv = np.ascontiguousarray(
            anw.transpose(0, 4, 1, 2, 3)).reshape(L, S, TL * 3 * S).astype(bf)

        small = np.zeros((L, S, NC_SM), f32)
        small[:, :, C_QB:C_QB + TL] = anb[:, :, 0, :].transpose(0, 2, 1)
        small[:, :, C_KB:C_KB + TL] = anb[:, :, 1, :].transpose(0, 2, 1)
        small[:, :, C_MT:C_MT + TL] = mask[:, sl].transpose(0, 2, 1)
        small[:, :, C_BVMT:C_BVMT + TL] = (anb[:, :, 2, :]
                                           * mask[:, sl]).transpose(0, 2, 1)
        small[:, :, C_WMT:C_WMT + TL] = wmat[:, sl].transpose(0, 2, 1)
        small[:, :, C_TC:C_TC + 3] = topo_c.transpose(0, 2, 1)
        small[:, :, C_TBP:C_TBP + 3] = topo_bp.transpose(0, 2, 1)
        small[:, :, C_GAM] = gamma
        small[:, :, C_BET] = beta
        small[1:, :, C_G0] = ada[:L - 1, :, 0]
        small[1:, :, C_G1H] = 0.5 * ada[:L - 1, :, 1]

        wbc = np.ascontiguousarray(wbias[:, sl].T)
        in_maps.append(dict(tqkv=tqkv, topo_wt=topo_wt_flat, small=small,
                            pre=pre, wbc=wbc, ident=ident, magic=magic,
                            selb=selb))
    return in_maps


def kernel(x, W, mask, attn_t, attn_n, attn_mask_n, norm_params, ada,
           span_ids, tb_ids):
    global _cached
    from concourse import bass_utils
    if _cached is None:
        _cached = _build()
    nc = _cached
    in_maps = _host_prep(x, W, mask, attn_t, attn_n, norm_params, ada)
    res = bass_utils.run_bass_kernel_spmd(nc, in_maps, core_ids=list(range(N_CORES)))
    out = np.concatenate([res.results[c]["out"].reshape(TL) for c in range(N_CORES)])
    return out.astype(np.float32)


# revision 4
# speedup vs baseline: 1.0130x; 1.0022x over previous
"""v3 Trainium2 Bass kernel.

Scheme (per topo batch b, SPMD over 8 cores, core c owns neurons
[c*16, (c+1)*16) of each batch):
  - v arrives as the DRAM AllGather of the previous batch's per-core
    affine outputs; gelu/ada applied on the gathered column.
  - topo norm stats via two 1-col matmuls + fast-inverse-sqrt Newton
    (2 iters) on [1,1] scalars; rstd/mu*rstd broadcast to columns with
    one ones-row matmul.
  - topo + neuron attention scores are built as PURE OUTER-PRODUCT
    matmuls: q and k*rs are computed as PSUM columns (stationary =
    bf16 weights, moving = 1-col activations, ~2ns each), bias-added,
    transposed once ([S,32] -> [32,S] via one bf16 ident matmul), and
    each neuron's score tile is stat=krs_row (x) moving=q_row
    (~107ns/neuron, bf16).  No broadcast-weight streams, no separate
    bias matmuls, no DVE scale pass: exp reads raw PSUM scores.
  - rs is folded into the K weights/biases host-side; gamma into the
    topo weights; v-bias*mask into a host column.
  - All PE operands bf16 (weights DMA'd in bf16: halves HBM traffic,
    4x fewer PE cycles per moving column).
  - exp per 512-col PSUM bank on ACT, E in bf16; PV = stat=E slice,
    moving = interleaved [v*m | m] bf16 pairs.
  - affine = ones-col matmul over wm*(zp+u') + bias row; result DMA'd
    to DRAM and AllGather'd (the 15us collective constant dominates
    each boundary; everything else is packed around it).
"""
import sys
import numpy as np
import ml_dtypes

sys.path.insert(0, "/opt/trn_rl_repo")

I, L, T, S = 128, 8, 128, 128
N_CORES = 8
TL = T // N_CORES
EPS = 1e-5
RS = float(1.0 / np.sqrt(np.float32(S)))
GC = 0.7978845608028654
GA = 0.044715
MAGIC = 0x5F3759DF

_cached = None

# small column layout
C_QB = 0          # 0:16   q bias cols
C_KB = 16         # 16:32  k bias cols (x rs)
C_MT = 32         # 32:48  mask cols
C_BVMT = 48       # 48:64  v-bias*mask cols
C_WMT = 64        # 64:80  (W*mask) cols
C_TC = 80         # 80:83  topo_c (k row x rs)
C_TBP = 83        # 83:86  topo_bp
C_GAM = 86
C_BET = 87
C_G0 = 88         # ada[b-1,:,0]
C_G1H = 89        # 0.5*ada[b-1,:,1]
NC_SM = 90


def _build():
    from concourse import bacc, tile, mybir

    fp32 = mybir.dt.float32
    bf16 = mybir.dt.bfloat16
    int32 = mybir.dt.int32
    Exp = mybir.ActivationFunctionType.Exp
    Tanh = mybir.ActivationFunctionType.Tanh
    mul_op = mybir.AluOpType.mult
    add_op = mybir.AluOpType.add
    sub_op = mybir.AluOpType.subtract
    shr_op = mybir.AluOpType.arith_shift_right
    bypass = mybir.AluOpType.bypass
    div_op = mybir.AluOpType.divide

    nc = bacc.Bacc("TRN2", target_bir_lowering=False, debug=False,
                   enable_asserts=True, num_devices=N_CORES)

    tqkv_d = nc.dram_tensor("tqkv", [L, S, TL * 3 * S], bf16,
                            kind="ExternalInput").ap()
    topo_wt_d = nc.dram_tensor("topo_wt", [L, S, 3 * S], bf16,
                               kind="ExternalInput").ap()
    small_d = nc.dram_tensor("small", [L, S, NC_SM], fp32,
                             kind="ExternalInput").ap()
    pre_d = nc.dram_tensor("pre", [S, 1], fp32, kind="ExternalInput").ap()
    wbc_d = nc.dram_tensor("wbc", [TL, L], fp32, kind="ExternalInput").ap()
    ident_d = nc.dram_tensor("ident", [S, S], bf16, kind="ExternalInput").ap()
    selb_d = nc.dram_tensor("selb", [TL, TL * S], bf16, kind="ExternalInput").ap()
    magic_d = nc.dram_tensor("magic", [1, 2], int32, kind="ExternalInput").ap()
    out_d = nc.dram_tensor("out", [TL, 1], fp32, kind="ExternalOutput").ap()

    with tile.TileContext(nc) as tc:
        with tc.tile_pool(name="wpool", bufs=3) as wpool, \
             tc.tile_pool(name="spool", bufs=3) as spool, \
             tc.tile_pool(name="fixed", bufs=1) as fixed, \
             tc.tile_pool(name="work", bufs=1) as work, \
             tc.tile_pool(name="ps_big", bufs=1, space="PSUM") as ps_big, \
             tc.tile_pool(name="ps_sc", bufs=2, space="PSUM") as ps_sc:

            identb = fixed.tile([S, S], bf16)
            nc.scalar.dma_start(identb[:], ident_d)
            selb = fixed.tile([TL, TL * S], bf16)
            nc.scalar.dma_start(selb[:], selb_d)
            pre = fixed.tile([S, 1], fp32)
            nc.scalar.dma_start(pre[:], pre_d)
            wbc = fixed.tile([TL, L], fp32)
            nc.scalar.dma_start(wbc[:], wbc_d)
            magic = fixed.tile([1, 2], int32)
            nc.scalar.dma_start(magic[:], magic_d)

            ones_col = fixed.tile([S, 1], fp32)
            nc.vector.memset(ones_col[:], 1.0)
            ones_colb = fixed.tile([S, 1], bf16)
            nc.vector.memset(ones_colb[:], 1.0)
            ones_row = fixed.tile([1, S], fp32)
            nc.vector.memset(ones_row[:], 1.0)
            ones_rowb = fixed.tile([1, S], bf16)
            nc.vector.memset(ones_rowb[:], 1.0)
            ones_matb = fixed.tile([S, S], bf16)
            nc.vector.memset(ones_matb[:], 1.0)

            v_bf = work.tile([S, 1], bf16)
            u_col = work.tile([S, 1], fp32)
            sc = work.tile([1, 8], fp32)
            sci = sc[:].bitcast(int32)
            yA = work.tile([1, 1], fp32)
            yB = work.tile([1, 1], fp32)
            yAi = yA[:].bitcast(int32)
            yBi = yB[:].bitcast(int32)
            bc_sb = work.tile([S, 2], fp32)
            qkvt = work.tile([S, 3], fp32)
            qb_t = work.tile([S, 1], bf16)
            qrow_t = work.tile([1, S], bf16)
            Et_sb = work.tile([S, S], bf16)
            pvr_t = work.tile([S, 2], bf16)
            nc.vector.memset(pvr_t[:], 1.0)
            rd1 = work.tile([S, 1], fp32)
            up_f = work.tile([S, 1], fp32)
            up_bf = work.tile([S, 1], bf16)
            qkb = work.tile([S, TL], bf16)
            krs_f = work.tile([S, TL], fp32)
            qT = work.tile([TL, S], bf16)
            sc_sb = work.tile([S, TL * S], fp32)
            vm1 = work.tile([S, TL], fp32)
            pvr = work.tile([S, 2 * TL], bf16)
            pvr2 = pvr[:].rearrange("p (t k) -> p t k", k=2)
            E_sb = work.tile([S, TL * S], bf16)
            rden = work.tile([S, TL], fp32)
            zp = work.tile([S, TL], fp32)
            zpu = work.tile([S, TL], fp32)
            afr = work.tile([S, TL], fp32)
            aff_sb = work.tile([TL, 1], fp32)

            sbank = [ps_big.tile([S, 512], fp32, name=f"sbank{i}")
                     for i in range(4)]

            ag_in = []
            ag_out = []
            for b in range(L - 1):
                ag_in.append(tc.tile([TL, 1], fp32, space="DRAM",
                                     name=f"agin{b}")[0])
                ag_out.append(tc.tile([S, 1], fp32, space="DRAM",
                                      addr_space="Shared", name=f"agout{b}")[0])

            def ts(out, in0, s1, op0, s2=None, op1=None):
                if s2 is None:
                    nc.vector.tensor_scalar(out, in0, s1, None, op0)
                else:
                    nc.vector.tensor_scalar(out, in0, s1, s2, op0, op1)

            def load_weights(b, eng, split=False):
                tq = wpool.tile([S, TL * 3 * S], bf16, tag="tq")
                if split:
                    HC = TL * 3 * S // 2
                    nc.scalar.dma_start(tq[:, 0:HC], tqkv_d[b][:, 0:HC])
                    eng.dma_start(tq[:, HC:2 * HC], tqkv_d[b][:, HC:2 * HC])
                else:
                    eng.dma_start(tq[:], tqkv_d[b])
                tw = spool.tile([S, 3 * S], bf16, tag="tw")
                eng.dma_start(tw[:], topo_wt_d[b])
                sm = spool.tile([S, NC_SM], fp32, tag="sm")
                eng.dma_start(sm[:], small_d[b])
                return tq, tw, sm

            wtiles = load_weights(0, nc.gpsimd, split=True)
            for b in range(L):
                tq, tw, sm = wtiles

                scr = ps_sc.tile([S, 512], fp32, tag="scr")
                qkv_ps = scr[:, 0:48]
                pvn_ps = scr[:, 48:80]
                aff_ps = scr[0:TL, 80:81]
                sv_ps = scr[0:1, 96:97]
                svv_ps = scr[0:1, 97:98]
                bc_ps = scr[:, 100:102]
                A_ps = scr[:, 104:107]
                pvt_ps = scr[:, 110:112]
                tr_ps = scr[0:TL, 128:256]
                trt_ps = scr[0:1, 256:384]
                tsc_ps = scr[:, 384:512]

                gam = sm[:, C_GAM:C_GAM + 1]
                bet = sm[:, C_BET:C_BET + 1]

                # ---- acquire v (gelu of previous batch's outputs) ----
                if b == 0:
                    nc.vector.tensor_copy(v_bf[:], pre[:, 0:1])
                else:
                    vin = work.tile([S, 1], fp32, tag="vin")
                    nc.sync.dma_start(vin[:], ag_out[b - 1][:])
                    g0 = sm[:, C_G0:C_G0 + 1]
                    g1h = sm[:, C_G1H:C_G1H + 1]
                    xg = work.tile([S, 1], fp32, tag="xg")
                    t1 = work.tile([S, 1], fp32, tag="t1")
                    t2 = work.tile([S, 1], fp32, tag="t2")
                    wg = work.tile([S, 1], fp32, tag="wg")
                    ts(xg[:], vin[:], g0, mul_op)
                    nc.vector.tensor_mul(t1[:], xg[:], xg[:])
                    ts(t1[:], t1[:], GA, mul_op, 1.0, add_op)
                    nc.vector.tensor_mul(t2[:], t1[:], xg[:])
                    nc.scalar.activation(t2[:], t2[:], Tanh, scale=GC)
                    ts(wg[:], xg[:], g1h, mul_op)
                    nc.vector.scalar_tensor_tensor(v_bf[:], t2[:], 1.0, wg[:],
                                                   add_op, mul_op)

                # ---- stats + Newton rsqrt (PE + DVE) ----
                nc.tensor.matmul(sv_ps, ones_colb[:], v_bf[:],
                                 start=True, stop=True)
                nc.tensor.matmul(svv_ps, v_bf[:], v_bf[:],
                                 start=True, stop=True)
                ts(sc[:, 0:1], sv_ps, 1.0 / S, mul_op)
                ts(sc[:, 1:2], svv_ps, 1.0 / S, mul_op)
                nc.vector.scalar_tensor_tensor(sc[:, 3:4], sc[:, 0:1],
                                               sc[:, 0:1], sc[:, 1:2],
                                               mul_op, sub_op)
                ts(sc[:, 4:5], sc[:, 3:4], -1.0, mul_op, EPS, add_op)
                ts(sc[:, 5:6], sc[:, 3:4], -0.5, mul_op, 0.5 * EPS, add_op)
                ts(yBi, sci[:, 4:5], 1, shr_op)
                nc.vector.tensor_sub(yAi, magic[:, 0:1], yBi)
                for _ in range(2):
                    nc.vector.scalar_tensor_tensor(yB[:], yA[:], sc[:, 5:6],
                                                   yA[:], mul_op, mul_op)
                    ts(yB[:], yB[:], -1.0, mul_op, 1.5, add_op)
                    nc.vector.tensor_mul(yA[:], yA[:], yB[:])
                nc.vector.tensor_copy(sc[:, 6:7], yA[:])
                nc.vector.tensor_mul(sc[:, 7:8], yA[:], sc[:, 0:1])
                nc.tensor.matmul(bc_ps, ones_row[:], sc[:, 6:8],
                                 start=True, stop=True)
                rstd_c = bc_ps[:, 0:1]
                murstd_c = bc_ps[:, 1:2]

                # ---- u = rstd*gamma*(v-mu) + beta ----
                gv = work.tile([S, 1], fp32, tag="gv")
                gm2 = work.tile([S, 1], fp32, tag="gm2")
                ts(gv[:], v_bf[:], gam, mul_op)
                ts(gm2[:], gam, murstd_c, mul_op, bet, sub_op)
                nc.vector.scalar_tensor_tensor(u_col[:], gv[:], rstd_c,
                                               gm2[:], mul_op, sub_op)

                # ---- topo qkv (gamma, rs folded host-side) ----
                for m in range(3):
                    nc.tensor.matmul(A_ps[:, m:m + 1], tw[:, m * S:(m + 1) * S],
                                     v_bf[:], start=True, stop=True)
                cm2 = work.tile([S, 3], fp32, tag="cm2")
                nc.vector.scalar_tensor_tensor(cm2[:], sm[:, C_TC:C_TC + 3],
                                               murstd_c, sm[:, C_TBP:C_TBP + 3],
                                               mul_op, sub_op)
                nc.vector.scalar_tensor_tensor(qkvt[:, 1:3], A_ps[:, 1:3],
                                               rstd_c, cm2[:, 1:3],
                                               mul_op, sub_op)
                qc_t = work.tile([S, 1], fp32, tag="qc_t")
                nc.vector.scalar_tensor_tensor(qc_t[:], A_ps[:, 0:1], rstd_c,
                                               cm2[:, 0:1], mul_op, sub_op)

                # ---- topo attention: q bcast via ones-mat, krs exp scale ----
                Ub_t = work.tile([S, S], bf16, tag="Ub_t")
                nc.vector.tensor_scalar(Ub_t[:], ones_matb[:], qc_t[:],
                                        None, mul_op)
                nc.vector.tensor_copy(pvr_t[:, 0:1], qkvt[:, 2:3])
                nc.tensor.matmul(tsc_ps, Ub_t[:], identb[:],
                                 start=True, stop=True)
                nc.scalar.activation(Et_sb[:], tsc_ps, Exp,
                                     scale=qkvt[:, 1:2])
                nc.tensor.matmul(pvt_ps, Et_sb[:], pvr_t[:],
                                 start=True, stop=True)
                nc.vector.reciprocal(rd1[:], pvt_ps[:, 1:2])
                nc.vector.scalar_tensor_tensor(up_bf[:], pvt_ps[:, 0:1],
                                               rd1[:], u_col[:],
                                               mul_op, add_op)
                nc.vector.scalar_tensor_tensor(up_f[:], pvt_ps[:, 0:1],
                                               rd1[:], u_col[:],
                                               mul_op, add_op)

                # ---- neuron q,k,v columns ----
                for tl in range(TL):
                    nc.tensor.matmul(qkv_ps[:, tl:tl + 1],
                                     tq[:, 3 * tl * S:(3 * tl + 1) * S],
                                     up_bf[:], start=True, stop=True)
                for tl in range(TL):
                    nc.tensor.matmul(qkv_ps[:, 16 + tl:17 + tl],
                                     tq[:, (3 * tl + 1) * S:(3 * tl + 2) * S],
                                     up_bf[:], start=True, stop=True)
                for tl in range(TL):
                    nc.tensor.matmul(qkv_ps[:, 32 + tl:33 + tl],
                                     tq[:, (3 * tl + 2) * S:(3 * tl + 3) * S],
                                     up_bf[:], start=True, stop=True)
                nc.vector.tensor_add(qkb[:], qkv_ps[:, 0:TL],
                                     sm[:, C_QB:C_QB + TL])
                nc.tensor.matmul(tr_ps, qkb[:], identb[:],
                                 start=True, stop=True)
                nc.vector.tensor_copy(qT[:], tr_ps)
                nc.vector.tensor_add(krs_f[:], qkv_ps[:, TL:2 * TL],
                                     sm[:, C_KB:C_KB + TL])
                nc.vector.tensor_mul(vm1[:], qkv_ps[:, 32:48],
                                     sm[:, C_MT:C_MT + TL])
                nc.vector.tensor_add(pvr2[:, :, 0], vm1[:],
                                     sm[:, C_BVMT:C_BVMT + TL])
                nc.vector.tensor_copy(pvr2[:, :, 1], sm[:, C_MT:C_MT + TL])

                # ---- scores: selector q-bcast; krs folded into the exp's
                # per-partition scale; PV per tl ----
                for tl in range(TL):
                    nc.tensor.matmul(sbank[tl // 4][:, (tl % 4) * S:(tl % 4 + 1) * S],
                                     selb[:, tl * S:(tl + 1) * S],
                                     qT[:], start=True, stop=True)
                for tl in range(8):
                    nc.scalar.activation(E_sb[:, tl * S:(tl + 1) * S],
                                         sbank[tl // 4][:, (tl % 4) * S:(tl % 4 + 1) * S],
                                         Exp, scale=krs_f[:, tl:tl + 1])
                for tl in range(8, TL):
                    ts(sc_sb[:, tl * S:(tl + 1) * S],
                       sbank[tl // 4][:, (tl % 4) * S:(tl % 4 + 1) * S],
                       krs_f[:, tl:tl + 1], mul_op)
                for bank in range(2, 4):
                    nc.scalar.activation(E_sb[:, bank * 512:(bank + 1) * 512],
                                         sc_sb[:, bank * 512:(bank + 1) * 512],
                                         Exp)
                for tl in range(TL):
                    nc.tensor.matmul(pvn_ps[:, 2 * tl:2 * tl + 2],
                                     E_sb[:, tl * S:(tl + 1) * S],
                                     pvr[:, 2 * tl:2 * tl + 2],
                                     start=True, stop=True)

                # ---- affine + output ----
                pv2 = pvn_ps.rearrange("p (t k) -> p t k", k=2)
                nc.vector.reciprocal(rden[:], pv2[:, :, 1])
                nc.vector.tensor_mul(zp[:], pv2[:, :, 0], rden[:])
                nc.vector.scalar_tensor_tensor(afr[:], zp[:], up_f[:],
                                               sm[:, C_WMT:C_WMT + TL],
                                               add_op, mul_op)
                nc.tensor.matmul(aff_ps, afr[:], ones_col[:],
                                 start=True, stop=True)
                nc.vector.tensor_add(aff_sb[:], aff_ps,
                                     wbc[:, b:b + 1])

                if b < L - 1:
                    nc.sync.dma_start(ag_in[b][:], aff_sb[:])
                    nc.gpsimd.collective_compute(
                        "AllGather", bypass,
                        replica_groups=[list(range(N_CORES))],
                        ins=[ag_in[b].opt()], outs=[ag_out[b].opt()],
                    )
                    wtiles = load_weights(b + 1, nc.gpsimd)
                else:
                    nc.sync.dma_start(out_d, aff_sb[:])

    nc.compile()
    return nc


def _host_prep(x, W, mask, attn_t, attn_n, norm_params, ada):
    f32 = np.float32
    bf = ml_dtypes.bfloat16
    x, W, mask, attn_t, attn_n, norm_params, ada = (
        np.ascontiguousarray(np.asarray(a, f32))
        for a in (x, W, mask, attn_t, attn_n, norm_params, ada))
    gamma = norm_params[:, 0, :]
    beta = norm_params[:, 1, :]

    rs_vec = np.array([1.0, RS, 1.0], f32)[None, :, None, None]  # scale k rows
    topo_w = attn_t[:, :, :, :S]
    topo_b = attn_t[:, :, :, S]
    topo_wg = topo_w * gamma[:, None, None, :] * rs_vec
    topo_wt_flat = np.ascontiguousarray(
        topo_wg.transpose(0, 3, 1, 2)).reshape(L, S, 3 * S).astype(bf)
    topo_c = topo_wg.sum(axis=3)                                   # (L,3,S)
    topo_bp = (np.einsum('lmis,ls->lmi', topo_w, beta) + topo_b) \
        * rs_vec[:, :, :, 0]

    wmat = W[:, :, :S] * mask
    wbias = W[:, :, S]

    ident = np.eye(S, dtype=bf)
    selb = np.zeros((TL, TL * S), dtype=bf)
    for tl in range(TL):
        selb[tl, tl * S:(tl + 1) * S] = 1.0
    magic = np.array([[MAGIC, 0]], np.int32)
    pre = np.ascontiguousarray(x.reshape(S, 1))

    in_maps = []
    for c in range(N_CORES):
        sl = slice(c * TL, (c + 1) * TL)
        an = attn_n[:, sl]
        anw = an[:, :, :, :, :S] * rs_vec[:, None, :, :, 0, None]  # (L,TL,3,j,i)
        anb = an[:, :, :, :, S] * rs_vec[:, None, :, :, 0]         # (L,TL,3,j)
        tqkv = np.ascontiguousarray(
            anw.transpose(0, 4, 1, 2, 3)).reshape(L, S, TL * 3 * S).astype(bf)

        small = np.zeros((L, S, NC_SM), f32)
        small[:, :, C_QB:C_QB + TL] = anb[:, :, 0, :].transpose(0, 2, 1)
        small[:, :, C_KB:C_KB + TL] = anb[:, :, 1, :].transpose(0, 2, 1)
        small[:, :, C_MT:C_MT + TL] = mask[:, sl].transpose(0, 2, 1)
        small[:, :, C_BVMT:C_BVMT + TL] = (anb[:, :, 2, :]
                                           * mask[:, sl]).transpose(0, 2, 1)
        small[:, :, C_WMT:C_WMT + TL] = wmat[:, sl].transpose(0, 2, 1)
        small[:, :, C_TC:C_TC + 3] = topo_c.transpose(0, 2, 1)
        small[:, :, C_TBP:C_TBP + 3] = topo_bp.transpose(0, 2, 1)
        small[:, :, C_GAM] = gamma
        small[:, :, C_BET] = beta
        small[1:, :, C_G0] = ada[:L - 1, :, 0]
        small[1:, :, C_G1H] = 0.5 * ada[:L - 1, :, 1]

        wbc = np.ascontiguousarray(wbias[:, sl].T)
        in_maps.append(dict(tqkv=tqkv, topo_wt=topo_wt_flat, small=small,
                            pre=pre, wbc=wbc, ident=ident, magic=magic,
                            selb=selb))
    return in_maps


def kernel(x, W, mask, attn_t, attn_n, attn_mask_n, norm_params, ada,
           span_ids, tb_ids):
    global _cached
    from concourse import bass_utils
    if _cached is None:
        _cached = _build()
    nc = _cached
    in_maps = _host_prep(x, W, mask, attn_t, attn_n, norm_params, ada)
    res = bass_utils.run_bass_kernel_spmd(nc, in_maps, core_ids=list(range(N_CORES)))
    out = np.concatenate([res.results[c]["out"].reshape(TL) for c in range(N_CORES)])
    return out.astype(np.float32)
